# revision 35
# baseline (speedup 1.0000x reference)
"""Trainium2 Bass kernel for BilinearDiscriminator.

Computes sigmoid((x*mask_x) @ W.T @ (y*mask_y).T) for x,y [8192,512],
W [512,512] -> out [8192,8192] fp32, SPMD across 8 NeuronCores.

Sharding: 8x1 row-parallel (a 4x2 grid would duplicate mm1 on every
m-column; 8x1 halves mm1's PE work). Core c handles rows
[c*1024, (c+1)*1024) of x; W and y are replicated. Host pre-applies the
dropout masks and hi/lo-quantizes the inputs.

BOTH matmuls run in fp8-e4m3 DoubleRow perf mode (0.5 cyc/row, K=256
per matmul -> 4x the fp16 row rate) with a 3-term hi/lo decomposition
to stay inside the 2e-2 error gate: for operands A ~= Ah + Al and
B ~= Bh + Bl (each an e4m3 value plus an e4m3-quantized residual),
  A@B ~= Ah@Bh + Ah@Bl + Al@Bh          (the dropped Al@Bl is O(eps^2))
costs 6 DoubleRow matmuls (1536 cyc) per [128 x 1024] output tile vs
fp16's 4 matmuls (2048 cyc): mm2 drops 109.2us -> 81.9us and mm1
6.8us -> 5.1us of PE time. Measured rel err 3.3e-3 vs the 2e-2 gate
(fp16 chain was 8e-4). W ships pre-scaled by 2^6 -- unscaled, its lo
residual (~1.2e-3) sits below e4m3's min subnormal and flushes to
zero, which collapses the correction term -- and the xt casts unscale
by 1/64 (Copy activation scale / scalar_tensor_tensor).

y's hi/lo pair is quantized on the host (same DMA bytes as fp16 y);
xt's pair is built on-chip from mm1's fp32 PSUM: xh = Copy-activation
cast on ACT (all sigmoids share one act-table set with Copy; a dummy
1-col sigmoid at t~0 pins that set so no 1.28us mid-program reload),
xl = scalar_tensor_tensor on DVE. Cast outputs are split into small
per-kc-pair tiles (xh0a/xh0b/xl0a/xl0b/xh1/xl1): the tile framework
turns dependencies into engine-lane count waits computed from emission
order, so coarse tiles (or casts emitted too early) serialize mm2's
first tiles behind unrelated casts.

Timeline-model notes that shaped the schedule:
  - PE p-state ramp is time-based from the first PE activity; warm-up
    dummy matmuls (Pool-memset operand, ~0.94us) hold the ramp stretch
    so mm1/mm2 run at 2.4GHz from ~3.9us.
  - wx8T packs Wh|xdh0|Wl|xdl0|xdh1|xdl1 fp8 column groups so six DMA
    pieces arrive in exact consumption order of mm1's dp-outer 3-term
    loops; mm1 starts at ~3.6us off the first (Wh|xdh) piece. More
    pieces would NOT help: HWDGE costs 625ns/DMA and the exclusive DMA
    device serializes transfers, so the last y-mb0 byte (~10.2us) is a
    hard floor for mm2's streaming start (~9.6us here).
  - PSUM: psum1 = 2x[128,512] bufs + psum2 = 3x[128,1024] bufs. mm1
    packs two accumulators per psum2 tile (bank-halves as separate
    accumulation groups; sub-ranges of one 2KB zero region must close
    before a sibling group starts). mm2 rotates psum2 3-deep so the
    1024-wide sigmoid (1038ns on ACT) never back-pressures the PE.
  - mm2 streams [128n x 1024m] tiles mb-major: 6 DR matmuls ordered
    xh.yh j0/j1, xl.yh j0, xh.yl j0/j1, xl.yh j1 (latest producers
    consumed last), one sigmoid PSUM->fp16 SBUF, one store. SIG_BUFS=28
    because stores queue ~20 tiles behind the y loads on the exclusive
    DMA device.
  - Tail: mb7's last five n-chunks taper to 512-wide pieces (sigmoid
    612ns < PE 640ns per piece, so ACT never backlogs into the close);
    the last three n-chunks' stores ride SWDGE prepare+trigger
    (kv_writeback): desc-gen runs early against a decoy source, each
    trigger fires after its sigmoid, skipping HWDGE+DGE on the
    program-closing chains. _rewire_prep_sems/_patch_prep_src adapt
    the prepared stores to the Tile framework.

Measured: 94534 ns (TimelineSim) vs 122032 ns fp16 ancestor (1.29x),
rel err 3.35e-3. Roughly: 0.9 start + 9.6 mm1+casts lead-in (y-DMA
bound) + 81.9 mm2 (PE-bound, zero steady-state stalls) + 2.8 close
(trailing sigmoid + final store sem + drain barriers).
"""

import os
import sys

sys.path.insert(0, "/opt/trn_rl_repo")

import ml_dtypes
import numpy as np

import concourse.bass as bass
import concourse.mybir as mybir
import concourse.tile as tile
from concourse import bacc
from concourse.bass_utils import run_bass_kernel_spmd

P = 128
N, M, D = 8192, 8192, 512
GRID = 8
N_LOC = N // GRID  # 1024
DC = D // P  # 4 chunks of the contraction dims
MB = 1024  # mm2 column block (2 PSUM banks)
WX = D + N_LOC  # packed W|xd row length (fp16 ancestor)
WX8 = 6 * 512  # packed fp8 Wh|xdh0|Wl|xdl0|xdh1|xdl1 row length

F8 = mybir.dt.float8e4
F16 = mybir.dt.float16
F32 = mybir.dt.float32
NP_F8 = ml_dtypes.float8_e4m3

DR = mybir.MatmulPerfMode.DoubleRow
# W is shipped pre-scaled by 2^6 so its e4m3 hi AND lo parts stay in the
# normal range (unscaled, the lo residual ~1.2e-3 sits below e4m3's min
# subnormal and flushes to zero); the xt casts unscale by 1/64.
W_SCALE = 64.0
W_INV = 1.0 / W_SCALE

_SIG_BUFS = int(os.environ.get("SIG_BUFS", "28"))
_PSUM1_BUFS = int(os.environ.get("PSUM1_BUFS", "2"))
_PSUM2_BUFS = int(os.environ.get("PSUM2_BUFS", "3"))
_N_WARM = int(os.environ.get("N_WARM", "24"))


def _build():
    nc = bacc.Bacc("TRN2", target_bir_lowering=False, debug=False)

    # wx8T packs the fp8 hi/lo pairs of W^T and xd^T row-wise so few DMAs
    # deliver mm1's operands in consumption order:
    # cols [Wh 0:512 | xdh-nt0 512:1024 | Wl 1024:1536 | xdl-nt0 1536:2048
    #       | xdh-nt1 2048:2560 | xdl-nt1 2560:3072].
    wx8T = nc.dram_tensor("wx8T", [D, WX8], F8, kind="ExternalInput").ap()
    ydhT = nc.dram_tensor("ydhT", [D, M], F8, kind="ExternalInput").ap()
    ydlT = nc.dram_tensor("ydlT", [D, M], F8, kind="ExternalInput").ap()
    out = nc.dram_tensor("out", [N_LOC, M], F16, kind="ExternalOutput").ap()

    # [ (dc p) cols ] -> [ p dc cols ] so one DMA can fill a multi-d-chunk
    # SBUF tile slice in partition-major order.
    wx8T_r = wx8T.rearrange("(dc p) c -> p dc c", p=P)
    ydhT_r = ydhT.rearrange("(dc p) m -> p dc m", p=P)
    ydlT_r = ydlT.rearrange("(dc p) m -> p dc m", p=P)

    with tile.TileContext(nc) as tc:
        with (
            tc.tile_pool(name="persist", bufs=1) as persist,
            tc.tile_pool(name="sig", bufs=_SIG_BUFS) as sigp,
        ):
            wx8 = persist.tile([P, DC, WX8], F8, name="wx8")
            ydh = persist.tile([P, DC, M], F8, name="ydh")
            ydl = persist.tile([P, DC, M], F8, name="ydl")
            # xt hi/lo in fp8: [p, kc, n] with semantic k = kc*128 + p.
            # Separate tiles per mm1 column half (n 0:512 vs 512:1024) AND,
            # for nt0, per kc pair: the dep tracker is tile-granular enough
            # that one big tile would make every mm2 tile's reads wait on
            # the LAST cast, not just its own kc rows' writers.
            xh0a = persist.tile([P, 2, 512], F8, name="xh0a")
            xl0a = persist.tile([P, 2, 512], F8, name="xl0a")
            xh0b = persist.tile([P, 2, 512], F8, name="xh0b")
            xl0b = persist.tile([P, 2, 512], F8, name="xl0b")
            xh1 = persist.tile([P, DC, 512], F8, name="xh1")
            xl1 = persist.tile([P, DC, 512], F8, name="xl1")
            # Warm-up operand: a small tile memset emitted FIRST on the
            # Pool queue (its framework memsets clear by ~440ns, before the
            # DVE's ~700ns startup barrier) so the first dummy matmul issues
            # as early as possible -- that's where the PE p-state threshold
            # anchors.
            warm = persist.tile([P, P], F16, name="warm")
            nc.gpsimd.memset(warm[:], 0.0)
            # ctx index for the final kv_writeback store (the out view is
            # pre-offset to the last 512 columns, so the index is 0).
            ctx_idx = persist.tile([P, 1], mybir.dt.int32, name="ctx_idx")
            nc.gpsimd.memset(ctx_idx[:], 0)
            # Decoy source for the kv_writeback prep: same geometry as a sig
            # tile, written once at t~0 so the prep's desc-gen has no late
            # deps. After the tile passes, the prep's source AP is patched to
            # the real final sig tile (see _patch_prep_src).
            sigf = persist.tile([P, MB], F16, name="sigf")
            nc.vector.memset(sigf[:], 0.0)
            # Dummy 1-column sigmoid emitted before any other activation:
            # the act-table pass then loads the set containing BOTH Sigmoid
            # and Copy ("sigmoid_and_others") once at t~0.6us on the idle
            # ACT engine, so neither the mm1 Copy casts nor the first mm2
            # sigmoid pays the 1.28us mid-program table reload. Own scratch
            # tile: the warm-up matmuls must not chain behind the table
            # load.
            scr = persist.tile([P, 1], F16, name="scr")
            nc.vector.memset(scr[:], 0.0)
            nc.scalar.activation(
                scr[:], scr[:], mybir.ActivationFunctionType.Sigmoid,
            )

            ctx_psum1 = tc.tile_pool(name="psum1", bufs=_PSUM1_BUFS, space="PSUM")
            psum1 = ctx_psum1.__enter__()
            ctx_psum2 = tc.tile_pool(name="psum2", bufs=_PSUM2_BUFS, space="PSUM")
            psum2 = ctx_psum2.__enter__()

            # Warm-up: dependency-free dummy matmuls occupy the PE's ~3us
            # p-state ramp window so the real matmuls, gated on their first
            # DMA, run at full clock. They rotate through psum2, idle until
            # mm2.
            for i in range(_N_WARM):
                pw = psum2.tile([P, MB], F32, name="ps2")
                nc.tensor.matmul(
                    pw[:, :P],
                    lhsT=warm[:, :P],
                    rhs=warm[:, :P],
                    start=True,
                    stop=True,
                )

            # Input DMAs in priority order: per-d-pair (Wh|xdh) then
            # (Wl|xdl) pieces gate mm1-nt0's dp-outer/term-inner loop in
            # exactly consumption order; the nt1 pieces follow; then y.
            for dp in range(DC // 2):
                for c0 in (0, 1024):
                    nc.sync.dma_start(
                        out=wx8[:, 2 * dp : 2 * dp + 2, c0 : c0 + 1024],
                        in_=wx8T_r[:, 2 * dp : 2 * dp + 2, c0 : c0 + 1024],
                    )
            for dp in range(DC // 2):
                nc.sync.dma_start(
                    out=wx8[:, 2 * dp : 2 * dp + 2, 2048:3072],
                    in_=wx8T_r[:, 2 * dp : 2 * dp + 2, 2048:3072],
                )
            # y-mb0 split into 512-column pieces, hi/lo interleaved: tile0
            # consumes yh[0:512] in its first matmuls, yl[0:512] two matmuls
            # later, yh[512:1024] in its second half -- this order gets each
            # piece's completion sem in just ahead of its first consumer.
            for t8, c0 in ((ydh, 0), (ydl, 0), (ydh, 512), (ydl, 512)):
                src8 = ydhT_r if t8 is ydh else ydlT_r
                nc.sync.dma_start(
                    out=t8[:, :, c0 : c0 + 512], in_=src8[:, :, c0 : c0 + 512]
                )
            for mb in range(1, M // MB):
                nc.sync.dma_start(
                    out=ydh[:, :, mb * MB : (mb + 1) * MB],
                    in_=ydhT_r[:, :, mb * MB : (mb + 1) * MB],
                )
                nc.sync.dma_start(
                    out=ydl[:, :, mb * MB : (mb + 1) * MB],
                    in_=ydlT_r[:, :, mb * MB : (mb + 1) * MB],
                )

            # SWDGE prepares for the final stores: desc-gen runs now (Pool is
            # idle; sources are the early memsets), each transfer fires at
            # its trigger after the producing ACT. Skips a normal DMA's
            # HWDGE+DGE stages on the program-closing chains.
            import bass_rust as _br

            def wb_prep(nchunk, col0, width):
                ov = out[
                    nchunk * P : (nchunk + 1) * P, col0 : col0 + width
                ].rearrange("(b p) (o m) -> b p o m", b=1, o=1)
                # The dho dim has count 1; kv_writeback asserts
                # row_stride == dho_count * dho_stride, so patch the
                # (addressing-irrelevant) stride of that dim.
                _dims = [list(d) for d in ov.ap]
                _dims[2] = [_dims[1][0], _dims[2][1]]
                ov.ap = _br.VecI64Pair(_dims)
                in_wb = sigf[:, 0:width].rearrange(
                    "p (o b n) -> p o b n", o=1, b=1
                )
                sem = nc.alloc_semaphore(f"kvwb_dma{nchunk}_{col0}")
                return nc.gpsimd.kv_writeback(
                    ov, in_wb, ctx_idx[:],
                    prepare_only=True, sem=sem, queue_num=0,
                )

            # All preps on queue 0; the count=1 triggers fire them in FIFO
            # order (prep emission order here must match trigger emission
            # order below). The tail tapers: n5 as 2x512, n6/n7 as 4x256
            # each, all on the prepared-store path -- smaller pieces let the
            # ACT engine chase the PE through the final matmuls instead of
            # serializing ~4us of sigmoid work after them.
            n_nc = N_LOC // P
            wb_keys = [
                "act_n5_0", "act_n5_1", "act_n6_0", "act_n6_1",
                "act_n7_0", "act_n7_1",
            ]
            wb_preps = [
                wb_prep(n_nc - 3, M - 1024, 512),
                wb_prep(n_nc - 3, M - 512, 512),
                wb_prep(n_nc - 2, M - 1024, 512),
                wb_prep(n_nc - 2, M - 512, 512),
                wb_prep(n_nc - 1, M - 1024, 512),
                wb_prep(n_nc - 1, M - 512, 512),
            ]

            # mm1 (fp8 DoubleRow, 3-term like mm2): xt ~= Wh@xdh + Wl@xdh
            # + Wh@xdl. nt0 (cols 0:512) runs dp-outer (dp = contraction
            # ktile pair) x term-inner so the first matmuls need only the
            # first (Wh|xdh) DMA piece; all 4 kc accumulators live at once
            # -- kc0/kc1 in the two psum1 bufs, kc2/kc3 packed into the two
            # bank-halves of one psum2 tile.
            T1, T2, T3 = (0, 512), (1024, 512), (0, 1536)  # (lhs, rhs) col0
            psA = psum2.tile([P, MB], F32, name="ps2")
            p1a = psum1.tile([P, 512], F32, name="ps1")
            p1b = psum1.tile([P, 512], F32, name="ps1")
            nt0_ps = {
                0: (p1a, 0),
                1: (p1b, 0),
                2: (psA, 0),
                3: (psA, 512),
            }
            for dp in range(DC // 2):
                dsl = slice(2 * dp, 2 * dp + 2)
                for ti, (lb, rb) in enumerate((T1, T2, T3)):
                    for kc in range(DC):
                        ps, o = nt0_ps[kc]
                        nc.tensor.matmul(
                            ps[:, o : o + 512],
                            lhsT=wx8[:, dsl, lb + kc * P : lb + (kc + 1) * P],
                            rhs=wx8[:, dsl, rb : rb + 512],
                            start=(dp == 0 and ti == 0),
                            stop=(dp == DC // 2 - 1 and ti == 2),
                            perf_mode=DR,
                        )
            # hi/lo casts: xh = fp8(ps) as a Copy activation on the
            # otherwise-idle ACT engine, xl = fp8(ps - xh) on DVE -- the two
            # streams pipeline, halving the cast chain that gates both
            # mm1-nt1's PSUM reuse and mm2's start. kc0/kc1 first: nt1
            # reuses their psum1 bufs.
            STT = nc.vector.scalar_tensor_tensor
            MUL = mybir.AluOpType.mult
            SUB = mybir.AluOpType.subtract
            # nt0 cast schedule, balancing the serial ACT and DVE chains so
            # the last xt-low producer lands earliest: the wide fused
            # kc2/kc3 copy goes FIRST on ACT (it gates the longest sub),
            # kc0's copy runs on DVE in parallel, and the subs chain on DVE
            # in c-completion order.
            xh0_23 = xh0b[:, :, :].rearrange("p a b -> p (a b)")
            xl0_23 = xl0b[:, :, :].rearrange("p a b -> p (a b)")
            nc.scalar.activation(
                xh0_23, psA[:, 0:1024], mybir.ActivationFunctionType.Copy,
                scale=W_INV,
            )
            nc.vector.tensor_scalar_mul(xh0a[:, 0, :], p1a[:], W_INV)
            STT(xl0a[:, 0, :], p1a[:], W_INV, xh0a[:, 0, :], MUL, SUB)
            nc.scalar.activation(
                xh0a[:, 1, :], p1b[:],
                mybir.ActivationFunctionType.Copy, scale=W_INV,
            )
            STT(xl0_23, psA[:, 0:1024], W_INV, xh0_23, MUL, SUB)
            STT(xl0a[:, 1, :], p1b[:], W_INV, xh0a[:, 1, :], MUL, SUB)

            # mm1, nt1 (cols 512:1024), kc-outer, as 384- then 128-column
            # sub-chunks matching the split DMAs above. The two sub-chunks
            # pack into ONE [128,512] accumulator range per kc (cols [0:384]
            # and [384:512] are separate accumulation groups) which maps
            # contiguously onto xt cols 512:1024, so each kc needs one
            # copy+sub. kc0/kc1 take the second psum2 tile's halves (free
            # since the warm-up dummies); kc2/kc3 reuse psum1 whose nt0
            # (kc0/kc1) casts complete earliest.
            psB = psum2.tile([P, MB], F32, name="ps2")
            p1c = psum1.tile([P, 512], F32, name="ps1")
            p1d = psum1.tile([P, 512], F32, name="ps1")
            nt1_ps = {
                0: (psB, 0),
                1: (psB, 512),
                2: (p1c, 0),
                3: (p1d, 0),
            }
            NT1_T = ((0, 2048), (1024, 2048), (0, 2560))  # (lhs, rhs) col0
            R0, R1 = (0, 384, 0), (384, 128, 384)

            def nt1_mm(kc, rng, dps):
                ps, po = nt1_ps[kc]
                xo, w, o = rng
                for dp in dps:
                    dsl = slice(2 * dp, 2 * dp + 2)
                    for ti, (lb, rb) in enumerate(NT1_T):
                        nc.tensor.matmul(
                            ps[:, po + o : po + o + w],
                            lhsT=wx8[:, dsl, lb + kc * P : lb + (kc + 1) * P],
                            rhs=wx8[:, dsl, rb + xo : rb + xo + w],
                            start=(dp == 0 and ti == 0),
                            stop=(dp == DC // 2 - 1 and ti == 2),
                            perf_mode=DR,
                        )

            # Group order absorbs the second nt1 DMA piece's arrival (kc0/
            # kc1's dp0 work runs first) and the psum1 WAR gates (kc2/kc3
            # wait the nt0-kc0/kc1 subs). Each (kc, range) accumulation
            # group still closes before its sibling range starts (shared
            # PSUM zero region).
            nt1_mm(0, R0, [0]); nt1_mm(1, R0, [0])
            nt1_mm(0, R0, [1]); nt1_mm(0, R1, [0, 1])
            nt1_mm(1, R0, [1]); nt1_mm(1, R1, [0, 1])
            for kc in (2, 3):
                nt1_mm(kc, R0, [0]); nt1_mm(kc, R0, [1])
                nt1_mm(kc, R1, [0, 1])
            # nt1 casts are EMITTED after mm2's first n-chunk (below):
            # the framework turns deps into engine-lane count waits computed
            # from emission order, so casts emitted here would inflate the
            # first mm2 tile's DVE wait to include them.
            def nt1_casts():
                xh1_01 = xh1[:, 0:2, :].rearrange("p a b -> p (a b)")
                nc.scalar.activation(
                    xh1_01, psB[:, 0:1024],
                    mybir.ActivationFunctionType.Copy, scale=W_INV,
                )
                xl1_01 = xl1[:, 0:2, :].rearrange("p a b -> p (a b)")
                STT(xl1_01, psB[:, 0:1024], W_INV, xh1_01, MUL, SUB)
                for kc in (2, 3):
                    ps, po = nt1_ps[kc]
                    nc.scalar.activation(
                        xh1[:, kc, :], ps[:, po : po + 512],
                        mybir.ActivationFunctionType.Copy, scale=W_INV,
                    )
                    STT(xl1[:, kc, :], ps[:, po : po + 512], W_INV,
                        xh1[:, kc, :], MUL, SUB)

            # mm2 + sigmoid + store, streaming mb-major over y blocks.
            # Each tile: 6 fp8 DoubleRow matmuls (K=256 each) accumulating
            # xh@yh + xh@yl + xl@yh into PSUM fp32. Term order puts yl- and
            # xl-dependent matmuls later to relax their producers' deadlines.
            n_mb = M // MB
            handles = {}

            def mm2_tile(mb, nchunk, width, coff, pool=None, wb_key=None,
                         ps=None, psoff=0):
                if ps is None:
                    if pool is None:
                        ps = psum2.tile([P, MB], F32, name="ps2")
                    else:
                        ps = pool.tile([P, 512], F32, name="ps1")
                grp = min(width, 512)
                if nchunk < 4:
                    # nt0 halves: per-j tiles with local ktile index 0:2.
                    xh_j = {0: (xh0a, slice(0, 2)), 1: (xh0b, slice(0, 2))}
                    xl_j = {0: (xl0a, slice(0, 2)), 1: (xl0b, slice(0, 2))}
                else:
                    xh_j = {j: (xh1, slice(2 * j, 2 * j + 2)) for j in (0, 1)}
                    xl_j = {j: (xl1, slice(2 * j, 2 * j + 2)) for j in (0, 1)}
                nsl = slice((nchunk % 4) * P, (nchunk % 4 + 1) * P)
                sig = sigp.tile([P, MB], F16, name="sig")
                # mt-outer: the first tile's first-half matmuls run before
                # its second-half ones, covering the second y half-block's
                # slightly later arrival.
                for mt in range(width // grp):
                    msl = slice(
                        mb * MB + coff + mt * grp, mb * MB + coff + (mt + 1) * grp
                    )
                    osl = slice(mt * grp, (mt + 1) * grp)
                    # Order: xh.yh (j0,j1), xl.yh j0, xh.yl (j0,j1),
                    # xl.yh j1 LAST -- the kc2/3 low-part cast (s23) is the
                    # latest xt producer, so its consumer goes last.
                    seq = [
                        (xh_j, ydh, 0), (xh_j, ydh, 1), (xl_j, ydh, 0),
                        (xh_j, ydl, 0), (xh_j, ydl, 1), (xl_j, ydh, 1),
                    ]
                    for i, (lhsd, rhs, j) in enumerate(seq):
                        lhs, ksl = lhsd[j]
                        nc.tensor.matmul(
                            ps[:, psoff + osl.start : psoff + osl.stop],
                            lhsT=lhs[:, ksl, nsl],
                            rhs=rhs[:, 2 * j : 2 * j + 2, msl],
                            start=(i == 0),
                            stop=(i == len(seq) - 1),
                            perf_mode=DR,
                        )
                act = nc.scalar.activation(
                    sig[:, :width],
                    ps[:, psoff : psoff + width],
                    mybir.ActivationFunctionType.Sigmoid,
                )
                if wb_key is not None:
                    # Prepared-store path: fire this tile's SWDGE descriptors
                    # (signals_writable carries the WAW dep on the ACT above;
                    # the matching prep is repointed at this sig tile by
                    # _patch_prep_src).
                    handles[wb_key] = act
                    nc.gpsimd.trigger_dma(
                        count=1, queue_num=0,
                        signals_writable=[sig[:, :width]],
                    )
                else:
                    nc.sync.dma_start(
                        out=out[
                            nchunk * P : (nchunk + 1) * P,
                            mb * MB + coff : mb * MB + coff + width,
                        ],
                        in_=sig[:, :width],
                    )

            # Tail taper: the last four n-chunks of the last y block run as
            # progressively smaller pieces, alternating psum2/psum1 for a
            # 5-deep effective rotation; n5-n7 ride the prepared-store path.
            for mb in range(n_mb):
                for nchunk in range(n_nc):
                    if mb == 0 and nchunk == 0:
                        # First tile as two 512 halves SHARING one psum2
                        # tile: the framework hoists a tile's waits onto its
                        # first instruction, so a 1024-wide tile0 would idle
                        # on the last y-mb0 piece that only its second half
                        # needs; sharing one tile keeps the psum2 rotation
                        # aligned so (mb0,n1) lands on psA (casts done
                        # early), not psB (casts late).
                        ps0 = psum2.tile([P, MB], F32, name="ps2")
                        mm2_tile(mb, nchunk, 512, 0, ps=ps0)
                        mm2_tile(mb, nchunk, 512, 512, ps=ps0, psoff=512)
                    elif mb == 0 and nchunk == 1:
                        mm2_tile(mb, nchunk, MB, 0)
                        # nt1 casts emitted only now: early enough to precede
                        # (mb0,n2) which reuses psB's psum buf, late enough
                        # that neither tile0's nor this tile's hoisted
                        # DVE-count waits include the nt1 subs.
                        nt1_casts()
                    elif mb == n_mb - 1 and nchunk in (n_nc - 5, n_nc - 4):
                        # n3/n4 as 2x512 normal tiles: steps the ACT chain
                        # down from 1038ns sigmoids before the prepared tail
                        # so no backlog carries into the close.
                        mm2_tile(mb, nchunk, 512, 0)
                        mm2_tile(mb, nchunk, 512, 512, pool=psum1)
                    elif mb == n_mb - 1 and nchunk >= n_nc - 3 and nchunk < n_nc - 1:
                        # Uniform 512-wide prepared pieces from n5 on: each
                        # sigmoid (612ns) is shorter than its piece's PE time
                        # (640ns), so the ACT chain never backlogs into the
                        # close (a 1024-wide n5 sigmoid did).
                        nk = f"n{5 + (nchunk - (n_nc - 3))}"
                        mm2_tile(mb, nchunk, 512, 0, wb_key=f"act_{nk}_0")
                        mm2_tile(mb, nchunk, 512, 512, pool=psum1,
                                 wb_key=f"act_{nk}_1")
                    elif mb == n_mb - 1 and nchunk == n_nc - 1:
                        mm2_tile(mb, nchunk, 512, 0, wb_key="act_n7_0")
                        mm2_tile(mb, nchunk, 512, 512, pool=psum1,
                                 wb_key="act_n7_1")
                    else:
                        mm2_tile(mb, nchunk, MB, 0)

            ctx_psum2.__exit__(None, None, None)
            ctx_psum1.__exit__(None, None, None)

    _rewire_prep_sems(nc, [p.ins for p in wb_preps])
    for key, p in zip(wb_keys, wb_preps):
        _patch_prep_src(p.ins, handles[key].ins)
    nc.compile()
    return nc


def _patch_prep_src(prep, act):
    """Repoint the kv_writeback prep's source from the sigf decoy to the
    real final sig tile (same geometry; only the memory ref differs)."""
    src = prep.ins[0]
    ref = act.outs[0]
    assert str(src.memref).startswith("sigf"), src.memref
    assert str(ref.memref).startswith("sig_"), ref.memref
    assert src.offset == ref.offset, (src.offset, ref.offset)
    src.memref = ref.memref
    src.memsetref = ref.memsetref


def _rewire_prep_sems(nc, preps):
    """Point each kv_writeback prep's DMA-completion update at the DMASW
    lane semaphore the tile wait pass expects.

    Tile's clock pass schedules a gen_mode==1 SWDGE prep on a DMASW lane, so
    downstream end-of-program barriers wait on that lane's semaphore; but the
    auto then_inc attach skips preps (the descriptor carries the caller's
    `sem=` instead), leaving the lane sem orphaned -> deadlock. Rewrite each
    prep's OnUpdate[0] to target its orphaned lane sem (lanes are assigned
    round-robin in emission order, so sorted lane names match prep order).
    """
    fn = nc.m.functions[0]
    updated_ids = set()
    waited = {}  # sem id -> ant_name for DMASW waits
    for block in fn.blocks:
        for ins in block.instructions:
            si = ins.sync_info
            if not si:
                continue
            for u in si.on_update:
                updated_ids.add(u.id)
            for w in si.on_wait:
                nm = getattr(w, "ant_name", None)
                if nm and str(nm).startswith("DMASW"):
                    waited[w.id] = nm
    orphans = sorted(
        (i for i in waited if i not in updated_ids),
        key=lambda i: str(waited[i]),
        reverse=True,
    )
    assert len(orphans) == len(preps), (
        f"expected {len(preps)} orphaned DMASW sems, got "
        f"{[(i, waited[i]) for i in orphans]}"
    )
    for prep, oid in zip(preps, orphans):
        upd = prep.sync_info.on_update[0]
        upd.id = oid
        upd.ant_name = waited[oid]


_NC = {}


def _get_nc():
    if "nc" not in _NC:
        _NC["nc"] = _build()
    return _NC["nc"]


def kernel(x, y, mask_x, mask_y, W):
    x = np.asarray(x, dtype=np.float32)
    y = np.asarray(y, dtype=np.float32)
    mask_x = np.asarray(mask_x, dtype=np.float32)
    mask_y = np.asarray(mask_y, dtype=np.float32)
    W = np.asarray(W, dtype=np.float32)

    xdT = np.ascontiguousarray((x * mask_x).T)  # [D, N] fp32
    xdhT = xdT.astype(NP_F8)
    xdlT = (xdT - xdhT.astype(np.float32)).astype(NP_F8)
    wT = W.T.astype(np.float32) * W_SCALE
    whT = wT.astype(NP_F8)
    wlT = (wT - whT.astype(np.float32)).astype(NP_F8)
    ydT = np.ascontiguousarray((y * mask_y).T)  # [D, M] fp32
    ydhT = ydT.astype(NP_F8)
    ydlT = (ydT - ydhT.astype(np.float32)).astype(NP_F8)

    in_maps = []
    for c in range(GRID):
        s = slice(c * N_LOC, (c + 1) * N_LOC)
        wx8T = np.empty((D, WX8), dtype=NP_F8)
        wx8T[:, 0:512] = whT
        wx8T[:, 512:1024] = xdhT[:, s][:, 0:512]
        wx8T[:, 1024:1536] = wlT
        wx8T[:, 1536:2048] = xdlT[:, s][:, 0:512]
        wx8T[:, 2048:2560] = xdhT[:, s][:, 512:1024]
        wx8T[:, 2560:3072] = xdlT[:, s][:, 512:1024]
        in_maps.append({"wx8T": wx8T, "ydhT": ydhT, "ydlT": ydlT})

    res = run_bass_kernel_spmd(_get_nc(), in_maps, list(range(8)))

    out = np.empty((N, M), dtype=np.float32)
    for c in range(GRID):
        out[c * N_LOC : (c + 1) * N_LOC, :] = res.results[c]["out"].astype(
            np.float32
        )
    return out


# revision 39
# speedup vs baseline: 1.0003x; 1.0003x over previous
"""Trainium2 Bass kernel for BilinearDiscriminator.

Computes sigmoid((x*mask_x) @ W.T @ (y*mask_y).T) for x,y [8192,512],
W [512,512] -> out [8192,8192] fp32, SPMD across 8 NeuronCores.

Sharding: 8x1 row-parallel (a 4x2 grid would duplicate mm1 on every
m-column; 8x1 halves mm1's PE work). Core c handles rows
[c*1024, (c+1)*1024) of x; W and y are replicated. Host pre-applies the
dropout masks and hi/lo-quantizes the inputs.

BOTH matmuls run in fp8-e4m3 DoubleRow perf mode (0.5 cyc/row, K=256
per matmul -> 4x the fp16 row rate) with a 3-term hi/lo decomposition
to stay inside the 2e-2 error gate: for operands A ~= Ah + Al and
B ~= Bh + Bl (each an e4m3 value plus an e4m3-quantized residual),
  A@B ~= Ah@Bh + Ah@Bl + Al@Bh          (the dropped Al@Bl is O(eps^2))
costs 6 DoubleRow matmuls (1536 cyc) per [128 x 1024] output tile vs
fp16's 4 matmuls (2048 cyc): mm2 drops 109.2us -> 81.9us and mm1
6.8us -> 5.1us of PE time. Measured rel err 3.3e-3 vs the 2e-2 gate
(fp16 chain was 8e-4). W ships pre-scaled by 2^6 -- unscaled, its lo
residual (~1.2e-3) sits below e4m3's min subnormal and flushes to
zero, which collapses the correction term -- and the xt casts unscale
by 1/64 (Copy activation scale / scalar_tensor_tensor).

y's hi/lo pair is quantized on the host (same DMA bytes as fp16 y);
xt's pair is built on-chip from mm1's fp32 PSUM: xh = Copy-activation
cast on ACT (all sigmoids share one act-table set with Copy; a dummy
1-col sigmoid at t~0 pins that set so no 1.28us mid-program reload),
xl = scalar_tensor_tensor on DVE. Cast outputs are split into small
per-kc-pair tiles (xh0a/xh0b/xl0a/xl0b/xh1/xl1): the tile framework
turns dependencies into engine-lane count waits computed from emission
order, so coarse tiles (or casts emitted too early) serialize mm2's
first tiles behind unrelated casts.

Timeline-model notes that shaped the schedule:
  - PE p-state ramp is time-based from the first PE activity; warm-up
    dummy matmuls (Pool-memset operand, ~0.94us) hold the ramp stretch
    so mm1/mm2 run at 2.4GHz from ~3.9us.
  - wx8T packs Wh|xdh0|Wl|xdl0|xdh1|xdl1 fp8 column groups so six DMA
    pieces arrive in exact consumption order of mm1's dp-outer 3-term
    loops; mm1 starts at ~3.6us off the first (Wh|xdh) piece. More
    pieces would NOT help: HWDGE costs 625ns/DMA and the exclusive DMA
    device serializes transfers, so the last y-mb0 byte (~10.2us) is a
    hard floor for mm2's streaming start (~9.6us here).
  - PSUM: psum1 = 2x[128,512] bufs + psum2 = 3x[128,1024] bufs. mm1
    packs two accumulators per psum2 tile (bank-halves as separate
    accumulation groups; sub-ranges of one 2KB zero region must close
    before a sibling group starts). mm2 rotates psum2 3-deep so the
    1024-wide sigmoid (1038ns on ACT) never back-pressures the PE.
  - mm2 streams [128n x 1024m] tiles mb-major: 6 DR matmuls ordered
    xh.yh j0/j1, xl.yh j0, xh.yl j0/j1, xl.yh j1 (latest producers
    consumed last), one sigmoid PSUM->fp16 SBUF, one store. SIG_BUFS=28
    because stores queue ~20 tiles behind the y loads on the exclusive
    DMA device.
  - Tail: mb7's last five n-chunks taper to 512-wide pieces (sigmoid
    612ns < PE 640ns per piece, so ACT never backlogs into the close);
    the last three n-chunks' stores ride SWDGE prepare+trigger
    (kv_writeback): desc-gen runs early against a decoy source, each
    trigger fires after its sigmoid, skipping HWDGE+DGE on the
    program-closing chains. _rewire_prep_sems/_patch_prep_src adapt
    the prepared stores to the Tile framework.

Measured: 94534 ns (TimelineSim) vs 122032 ns fp16 ancestor (1.29x),
rel err 3.35e-3. Roughly: 0.9 start + 9.6 mm1+casts lead-in (y-DMA
bound) + 81.9 mm2 (PE-bound, zero steady-state stalls) + 2.8 close
(trailing sigmoid + final store sem + drain barriers).
"""

import os
import sys

sys.path.insert(0, "/opt/trn_rl_repo")

import ml_dtypes
import numpy as np

import concourse.bass as bass
import concourse.mybir as mybir
import concourse.tile as tile
from concourse import bacc
from concourse.bass_utils import run_bass_kernel_spmd

P = 128
N, M, D = 8192, 8192, 512
GRID = 8
N_LOC = N // GRID  # 1024
DC = D // P  # 4 chunks of the contraction dims
MB = 1024  # mm2 column block (2 PSUM banks)
WX = D + N_LOC  # packed W|xd row length (fp16 ancestor)
WX8 = 6 * 512  # packed fp8 Wh|xdh0|Wl|xdl0|xdh1|xdl1 row length

F8 = mybir.dt.float8e4
F16 = mybir.dt.float16
F32 = mybir.dt.float32
NP_F8 = ml_dtypes.float8_e4m3

DR = mybir.MatmulPerfMode.DoubleRow
# W is shipped pre-scaled by 2^6 so its e4m3 hi AND lo parts stay in the
# normal range (unscaled, the lo residual ~1.2e-3 sits below e4m3's min
# subnormal and flushes to zero); the xt casts unscale by 1/64.
W_SCALE = 64.0
W_INV = 1.0 / W_SCALE

_SIG_BUFS = int(os.environ.get("SIG_BUFS", "28"))
_PSUM1_BUFS = int(os.environ.get("PSUM1_BUFS", "2"))
_PSUM2_BUFS = int(os.environ.get("PSUM2_BUFS", "3"))
_N_WARM = int(os.environ.get("N_WARM", "24"))


def _build():
    nc = bacc.Bacc("TRN2", target_bir_lowering=False, debug=False)

    # wx8T packs the fp8 hi/lo pairs of W^T and xd^T row-wise so few DMAs
    # deliver mm1's operands in consumption order:
    # cols [Wh 0:512 | xdh-nt0 512:1024 | Wl 1024:1536 | xdl-nt0 1536:2048
    #       | xdh-nt1 2048:2560 | xdl-nt1 2560:3072].
    wx8T = nc.dram_tensor("wx8T", [D, WX8], F8, kind="ExternalInput").ap()
    ydhT = nc.dram_tensor("ydhT", [D, M], F8, kind="ExternalInput").ap()
    ydlT = nc.dram_tensor("ydlT", [D, M], F8, kind="ExternalInput").ap()
    out = nc.dram_tensor("out", [N_LOC, M], F16, kind="ExternalOutput").ap()

    # [ (dc p) cols ] -> [ p dc cols ] so one DMA can fill a multi-d-chunk
    # SBUF tile slice in partition-major order.
    wx8T_r = wx8T.rearrange("(dc p) c -> p dc c", p=P)
    ydhT_r = ydhT.rearrange("(dc p) m -> p dc m", p=P)
    ydlT_r = ydlT.rearrange("(dc p) m -> p dc m", p=P)

    with tile.TileContext(nc) as tc:
        with (
            tc.tile_pool(name="persist", bufs=1) as persist,
            tc.tile_pool(name="sig", bufs=_SIG_BUFS) as sigp,
        ):
            wx8 = persist.tile([P, DC, WX8], F8, name="wx8")
            ydh = persist.tile([P, DC, M], F8, name="ydh")
            ydl = persist.tile([P, DC, M], F8, name="ydl")
            # xt hi/lo in fp8: [p, kc, n] with semantic k = kc*128 + p.
            # Separate tiles per mm1 column half (n 0:512 vs 512:1024) AND,
            # for nt0, per kc pair: the dep tracker is tile-granular enough
            # that one big tile would make every mm2 tile's reads wait on
            # the LAST cast, not just its own kc rows' writers.
            xh0a = persist.tile([P, 2, 512], F8, name="xh0a")
            xl0a = persist.tile([P, 2, 512], F8, name="xl0a")
            xh0b = persist.tile([P, 2, 512], F8, name="xh0b")
            xl0b = persist.tile([P, 2, 512], F8, name="xl0b")
            xh1 = persist.tile([P, DC, 512], F8, name="xh1")
            xl1 = persist.tile([P, DC, 512], F8, name="xl1")
            # Warm-up operand: a small tile memset emitted FIRST on the
            # Pool queue (its framework memsets clear by ~440ns, before the
            # DVE's ~700ns startup barrier) so the first dummy matmul issues
            # as early as possible -- that's where the PE p-state threshold
            # anchors.
            warm = persist.tile([P, P], F16, name="warm")
            nc.gpsimd.memset(warm[:], 0.0)
            # ctx index for the final kv_writeback store (the out view is
            # pre-offset to the last 512 columns, so the index is 0).
            ctx_idx = persist.tile([P, 1], mybir.dt.int32, name="ctx_idx")
            nc.gpsimd.memset(ctx_idx[:], 0)
            # Decoy source for the kv_writeback prep: same geometry as a sig
            # tile, written once at t~0 so the prep's desc-gen has no late
            # deps. After the tile passes, the prep's source AP is patched to
            # the real final sig tile (see _patch_prep_src).
            sigf = persist.tile([P, MB], F16, name="sigf")
            nc.vector.memset(sigf[:], 0.0)
            # Dummy 1-column sigmoid emitted before any other activation:
            # the act-table pass then loads the set containing BOTH Sigmoid
            # and Copy ("sigmoid_and_others") once at t~0.6us on the idle
            # ACT engine, so neither the mm1 Copy casts nor the first mm2
            # sigmoid pays the 1.28us mid-program table reload. Own scratch
            # tile: the warm-up matmuls must not chain behind the table
            # load.
            scr = persist.tile([P, 1], F16, name="scr")
            nc.vector.memset(scr[:], 0.0)
            nc.scalar.activation(
                scr[:], scr[:], mybir.ActivationFunctionType.Sigmoid,
            )

            ctx_psum1 = tc.tile_pool(name="psum1", bufs=_PSUM1_BUFS, space="PSUM")
            psum1 = ctx_psum1.__enter__()
            ctx_psum2 = tc.tile_pool(name="psum2", bufs=_PSUM2_BUFS, space="PSUM")
            psum2 = ctx_psum2.__enter__()

            # Warm-up: dependency-free dummy matmuls occupy the PE's ~3us
            # p-state ramp window so the real matmuls, gated on their first
            # DMA, run at full clock. They rotate through psum2, idle until
            # mm2.
            for i in range(_N_WARM):
                pw = psum2.tile([P, MB], F32, name="ps2")
                nc.tensor.matmul(
                    pw[:, :P],
                    lhsT=warm[:, :P],
                    rhs=warm[:, :P],
                    start=True,
                    stop=True,
                )

            # Input DMAs in priority order: per-d-pair (Wh|xdh) then
            # (Wl|xdl) pieces gate mm1-nt0's dp-outer/term-inner loop in
            # exactly consumption order; the nt1 pieces follow; then y.
            for dp in range(DC // 2):
                for c0 in (0, 1024):
                    nc.sync.dma_start(
                        out=wx8[:, 2 * dp : 2 * dp + 2, c0 : c0 + 1024],
                        in_=wx8T_r[:, 2 * dp : 2 * dp + 2, c0 : c0 + 1024],
                    )
            for dp in range(DC // 2):
                nc.sync.dma_start(
                    out=wx8[:, 2 * dp : 2 * dp + 2, 2048:3072],
                    in_=wx8T_r[:, 2 * dp : 2 * dp + 2, 2048:3072],
                )
            # y-mb0 split into 512-column pieces, hi/lo interleaved: tile0
            # consumes yh[0:512] in its first matmuls, yl[0:512] two matmuls
            # later, yh[512:1024] in its second half -- this order gets each
            # piece's completion sem in just ahead of its first consumer.
            for t8, c0 in ((ydh, 0), (ydl, 0), (ydh, 512), (ydl, 512)):
                src8 = ydhT_r if t8 is ydh else ydlT_r
                nc.sync.dma_start(
                    out=t8[:, :, c0 : c0 + 512], in_=src8[:, :, c0 : c0 + 512]
                )
            for mb in range(1, M // MB):
                nc.sync.dma_start(
                    out=ydh[:, :, mb * MB : (mb + 1) * MB],
                    in_=ydhT_r[:, :, mb * MB : (mb + 1) * MB],
                )
                nc.sync.dma_start(
                    out=ydl[:, :, mb * MB : (mb + 1) * MB],
                    in_=ydlT_r[:, :, mb * MB : (mb + 1) * MB],
                )

            # SWDGE prepares for the final stores: desc-gen runs now (Pool is
            # idle; sources are the early memsets), each transfer fires at
            # its trigger after the producing ACT. Skips a normal DMA's
            # HWDGE+DGE stages on the program-closing chains.
            import bass_rust as _br

            def wb_prep(nchunk, col0, width):
                ov = out[
                    nchunk * P : (nchunk + 1) * P, col0 : col0 + width
                ].rearrange("(b p) (o m) -> b p o m", b=1, o=1)
                # The dho dim has count 1; kv_writeback asserts
                # row_stride == dho_count * dho_stride, so patch the
                # (addressing-irrelevant) stride of that dim.
                _dims = [list(d) for d in ov.ap]
                _dims[2] = [_dims[1][0], _dims[2][1]]
                ov.ap = _br.VecI64Pair(_dims)
                in_wb = sigf[:, 0:width].rearrange(
                    "p (o b n) -> p o b n", o=1, b=1
                )
                sem = nc.alloc_semaphore(f"kvwb_dma{nchunk}_{col0}")
                return nc.gpsimd.kv_writeback(
                    ov, in_wb, ctx_idx[:],
                    prepare_only=True, sem=sem, queue_num=0,
                )

            # All preps on queue 0; the count=1 triggers fire them in FIFO
            # order (prep emission order here must match trigger emission
            # order below). The tail tapers: n5 as 2x512, n6/n7 as 4x256
            # each, all on the prepared-store path -- smaller pieces let the
            # ACT engine chase the PE through the final matmuls instead of
            # serializing ~4us of sigmoid work after them.
            n_nc = N_LOC // P
            wb_keys = [
                "act_n5_0", "act_n5_1", "act_n6_0", "act_n6_1",
                "act_n7_0", "act_n7_1",
            ]
            wb_preps = [
                wb_prep(n_nc - 3, M - 1024, 512),
                wb_prep(n_nc - 3, M - 512, 512),
                wb_prep(n_nc - 2, M - 1024, 512),
                wb_prep(n_nc - 2, M - 512, 512),
                wb_prep(n_nc - 1, M - 1024, 512),
                wb_prep(n_nc - 1, M - 512, 512),
            ]

            # mm1 (fp8 DoubleRow, 3-term like mm2): xt ~= Wh@xdh + Wl@xdh
            # + Wh@xdl. nt0 (cols 0:512) runs dp-outer (dp = contraction
            # ktile pair) x term-inner so the first matmuls need only the
            # first (Wh|xdh) DMA piece; all 4 kc accumulators live at once
            # -- kc0/kc1 in the two psum1 bufs, kc2/kc3 packed into the two
            # bank-halves of one psum2 tile.
            T1, T2, T3 = (0, 512), (1024, 512), (0, 1536)  # (lhs, rhs) col0
            psA = psum2.tile([P, MB], F32, name="ps2")
            p1a = psum1.tile([P, 512], F32, name="ps1")
            p1b = psum1.tile([P, 512], F32, name="ps1")
            nt0_ps = {
                0: (p1a, 0),
                1: (p1b, 0),
                2: (psA, 0),
                3: (psA, 512),
            }
            for dp in range(DC // 2):
                dsl = slice(2 * dp, 2 * dp + 2)
                for ti, (lb, rb) in enumerate((T1, T2, T3)):
                    for kc in range(DC):
                        ps, o = nt0_ps[kc]
                        nc.tensor.matmul(
                            ps[:, o : o + 512],
                            lhsT=wx8[:, dsl, lb + kc * P : lb + (kc + 1) * P],
                            rhs=wx8[:, dsl, rb : rb + 512],
                            start=(dp == 0 and ti == 0),
                            stop=(dp == DC // 2 - 1 and ti == 2),
                            perf_mode=DR,
                        )
            # hi/lo casts: xh = fp8(ps) as a Copy activation on the
            # otherwise-idle ACT engine, xl = fp8(ps - xh) on DVE -- the two
            # streams pipeline, halving the cast chain that gates both
            # mm1-nt1's PSUM reuse and mm2's start. kc0/kc1 first: nt1
            # reuses their psum1 bufs.
            STT = nc.vector.scalar_tensor_tensor
            MUL = mybir.AluOpType.mult
            SUB = mybir.AluOpType.subtract
            # nt0 cast schedule, balancing the serial ACT and DVE chains so
            # the last xt-low producer lands earliest: the wide fused
            # kc2/kc3 copy goes FIRST on ACT (it gates the longest sub),
            # kc0's copy runs on DVE in parallel, and the subs chain on DVE
            # in c-completion order.
            xh0_23 = xh0b[:, :, :].rearrange("p a b -> p (a b)")
            xl0_23 = xl0b[:, :, :].rearrange("p a b -> p (a b)")
            nc.scalar.activation(
                xh0_23, psA[:, 0:1024], mybir.ActivationFunctionType.Copy,
                scale=W_INV,
            )
            nc.vector.tensor_scalar_mul(xh0a[:, 0, :], p1a[:], W_INV)
            STT(xl0a[:, 0, :], p1a[:], W_INV, xh0a[:, 0, :], MUL, SUB)
            nc.scalar.activation(
                xh0a[:, 1, :], p1b[:],
                mybir.ActivationFunctionType.Copy, scale=W_INV,
            )
            STT(xl0_23, psA[:, 0:1024], W_INV, xh0_23, MUL, SUB)
            STT(xl0a[:, 1, :], p1b[:], W_INV, xh0a[:, 1, :], MUL, SUB)

            # mm1, nt1 (cols 512:1024), kc-outer, as 384- then 128-column
            # sub-chunks matching the split DMAs above. The two sub-chunks
            # pack into ONE [128,512] accumulator range per kc (cols [0:384]
            # and [384:512] are separate accumulation groups) which maps
            # contiguously onto xt cols 512:1024, so each kc needs one
            # copy+sub. kc0/kc1 take the second psum2 tile's halves (free
            # since the warm-up dummies); kc2/kc3 reuse psum1 whose nt0
            # (kc0/kc1) casts complete earliest.
            psB = psum2.tile([P, MB], F32, name="ps2")
            p1c = psum1.tile([P, 512], F32, name="ps1")
            p1d = psum1.tile([P, 512], F32, name="ps1")
            nt1_ps = {
                0: (psB, 0),
                1: (psB, 512),
                2: (p1c, 0),
                3: (p1d, 0),
            }
            NT1_T = ((0, 2048), (1024, 2048), (0, 2560))  # (lhs, rhs) col0
            R0, R1 = (0, 384, 0), (384, 128, 384)

            def nt1_mm(kc, rng, dps):
                ps, po = nt1_ps[kc]
                xo, w, o = rng
                for dp in dps:
                    dsl = slice(2 * dp, 2 * dp + 2)
                    for ti, (lb, rb) in enumerate(NT1_T):
                        nc.tensor.matmul(
                            ps[:, po + o : po + o + w],
                            lhsT=wx8[:, dsl, lb + kc * P : lb + (kc + 1) * P],
                            rhs=wx8[:, dsl, rb + xo : rb + xo + w],
                            start=(dp == 0 and ti == 0),
                            stop=(dp == DC // 2 - 1 and ti == 2),
                            perf_mode=DR,
                        )

            # Group order absorbs the second nt1 DMA piece's arrival (kc0/
            # kc1's dp0 work runs first) and the psum1 WAR gates (kc2/kc3
            # wait the nt0-kc0/kc1 subs). Each (kc, range) accumulation
            # group still closes before its sibling range starts (shared
            # PSUM zero region).
            nt1_mm(0, R0, [0]); nt1_mm(1, R0, [0])
            nt1_mm(0, R0, [1]); nt1_mm(0, R1, [0, 1])
            nt1_mm(1, R0, [1]); nt1_mm(1, R1, [0, 1])
            for kc in (2, 3):
                nt1_mm(kc, R0, [0]); nt1_mm(kc, R0, [1])
                nt1_mm(kc, R1, [0, 1])
            # nt1 casts are EMITTED after mm2's first n-chunk (below):
            # the framework turns deps into engine-lane count waits computed
            # from emission order, so casts emitted here would inflate the
            # first mm2 tile's DVE wait to include them.
            def nt1_casts():
                xh1_01 = xh1[:, 0:2, :].rearrange("p a b -> p (a b)")
                nc.scalar.activation(
                    xh1_01, psB[:, 0:1024],
                    mybir.ActivationFunctionType.Copy, scale=W_INV,
                )
                xl1_01 = xl1[:, 0:2, :].rearrange("p a b -> p (a b)")
                STT(xl1_01, psB[:, 0:1024], W_INV, xh1_01, MUL, SUB)
                # kc2/kc3 copies on DVE, not ACT: on ACT the scheduler
                # runs them (ready early) ahead of mm2's first sigmoids,
                # and the n1 tile's hoisted ACT-count wait then spans them.
                for kc in (2, 3):
                    ps, po = nt1_ps[kc]
                    nc.vector.tensor_scalar_mul(
                        xh1[:, kc, :], ps[:, po : po + 512], W_INV
                    )
                    STT(xl1[:, kc, :], ps[:, po : po + 512], W_INV,
                        xh1[:, kc, :], MUL, SUB)

            # mm2 + sigmoid + store, streaming mb-major over y blocks.
            # Each tile: 6 fp8 DoubleRow matmuls (K=256 each) accumulating
            # xh@yh + xh@yl + xl@yh into PSUM fp32. Term order puts yl- and
            # xl-dependent matmuls later to relax their producers' deadlines.
            n_mb = M // MB
            handles = {}

            def mm2_tile(mb, nchunk, width, coff, pool=None, wb_key=None,
                         ps=None, psoff=0):
                if ps is None:
                    if pool is None:
                        ps = psum2.tile([P, MB], F32, name="ps2")
                    else:
                        ps = pool.tile([P, 512], F32, name="ps1")
                grp = min(width, 512)
                if nchunk < 4:
                    # nt0 halves: per-j tiles with local ktile index 0:2.
                    xh_j = {0: (xh0a, slice(0, 2)), 1: (xh0b, slice(0, 2))}
                    xl_j = {0: (xl0a, slice(0, 2)), 1: (xl0b, slice(0, 2))}
                else:
                    xh_j = {j: (xh1, slice(2 * j, 2 * j + 2)) for j in (0, 1)}
                    xl_j = {j: (xl1, slice(2 * j, 2 * j + 2)) for j in (0, 1)}
                nsl = slice((nchunk % 4) * P, (nchunk % 4 + 1) * P)
                sig = sigp.tile([P, MB], F16, name="sig")
                # mt-outer: the first tile's first-half matmuls run before
                # its second-half ones, covering the second y half-block's
                # slightly later arrival.
                for mt in range(width // grp):
                    msl = slice(
                        mb * MB + coff + mt * grp, mb * MB + coff + (mt + 1) * grp
                    )
                    osl = slice(mt * grp, (mt + 1) * grp)
                    # Order: xh.yh (j0,j1), xl.yh j0, xh.yl (j0,j1),
                    # xl.yh j1 LAST -- the kc2/3 low-part cast (s23) is the
                    # latest xt producer, so its consumer goes last.
                    seq = [
                        (xh_j, ydh, 0), (xh_j, ydh, 1), (xl_j, ydh, 0),
                        (xh_j, ydl, 0), (xh_j, ydl, 1), (xl_j, ydh, 1),
                    ]
                    for i, (lhsd, rhs, j) in enumerate(seq):
                        lhs, ksl = lhsd[j]
                        nc.tensor.matmul(
                            ps[:, psoff + osl.start : psoff + osl.stop],
                            lhsT=lhs[:, ksl, nsl],
                            rhs=rhs[:, 2 * j : 2 * j + 2, msl],
                            start=(i == 0),
                            stop=(i == len(seq) - 1),
                            perf_mode=DR,
                        )
                act = nc.scalar.activation(
                    sig[:, :width],
                    ps[:, psoff : psoff + width],
                    mybir.ActivationFunctionType.Sigmoid,
                )
                if wb_key is not None:
                    # Prepared-store path: fire this tile's SWDGE descriptors
                    # (signals_writable carries the WAW dep on the ACT above;
                    # the matching prep is repointed at this sig tile by
                    # _patch_prep_src).
                    handles[wb_key] = act
                    nc.gpsimd.trigger_dma(
                        count=1, queue_num=0,
                        signals_writable=[sig[:, :width]],
                    )
                else:
                    nc.sync.dma_start(
                        out=out[
                            nchunk * P : (nchunk + 1) * P,
                            mb * MB + coff : mb * MB + coff + width,
                        ],
                        in_=sig[:, :width],
                    )

            # Tail taper: the last four n-chunks of the last y block run as
            # progressively smaller pieces, alternating psum2/psum1 for a
            # 5-deep effective rotation; n5-n7 ride the prepared-store path.
            for mb in range(n_mb):
                for nchunk in range(n_nc):
                    if mb == 0 and nchunk == 0:
                        # First tile as two 512 halves SHARING one psum2
                        # tile: the framework hoists a tile's waits onto its
                        # first instruction, so a 1024-wide tile0 would idle
                        # on the last y-mb0 piece that only its second half
                        # needs; sharing one tile keeps the psum2 rotation
                        # aligned so (mb0,n1) lands on psA (casts done
                        # early), not psB (casts late).
                        ps0 = psum2.tile([P, MB], F32, name="ps2")
                        mm2_tile(mb, nchunk, 512, 0, ps=ps0)
                        mm2_tile(mb, nchunk, 512, 512, ps=ps0, psoff=512)
                    elif mb == 0 and nchunk == 1:
                        mm2_tile(mb, nchunk, MB, 0)
                        # nt1 casts emitted only now: early enough to precede
                        # (mb0,n2) which reuses psB's psum buf, late enough
                        # that neither tile0's nor this tile's hoisted
                        # DVE-count waits include the nt1 subs.
                        nt1_casts()
                    elif mb == n_mb - 1 and nchunk in (n_nc - 5, n_nc - 4):
                        # n3/n4 as 2x512 normal tiles: steps the ACT chain
                        # down from 1038ns sigmoids before the prepared tail
                        # so no backlog carries into the close.
                        mm2_tile(mb, nchunk, 512, 0)
                        mm2_tile(mb, nchunk, 512, 512, pool=psum1)
                    elif mb == n_mb - 1 and nchunk >= n_nc - 3 and nchunk < n_nc - 1:
                        # Uniform 512-wide prepared pieces from n5 on: each
                        # sigmoid (612ns) is shorter than its piece's PE time
                        # (640ns), so the ACT chain never backlogs into the
                        # close (a 1024-wide n5 sigmoid did).
                        nk = f"n{5 + (nchunk - (n_nc - 3))}"
                        mm2_tile(mb, nchunk, 512, 0, wb_key=f"act_{nk}_0")
                        mm2_tile(mb, nchunk, 512, 512, pool=psum1,
                                 wb_key=f"act_{nk}_1")
                    elif mb == n_mb - 1 and nchunk == n_nc - 1:
                        mm2_tile(mb, nchunk, 512, 0, wb_key="act_n7_0")
                        mm2_tile(mb, nchunk, 512, 512, pool=psum1,
                                 wb_key="act_n7_1")
                    else:
                        mm2_tile(mb, nchunk, MB, 0)

            ctx_psum2.__exit__(None, None, None)
            ctx_psum1.__exit__(None, None, None)

    _rewire_prep_sems(nc, [p.ins for p in wb_preps])
    for key, p in zip(wb_keys, wb_preps):
        _patch_prep_src(p.ins, handles[key].ins)
    nc.compile()
    return nc


def _patch_prep_src(prep, act):
    """Repoint the kv_writeback prep's source from the sigf decoy to the
    real final sig tile (same geometry; only the memory ref differs)."""
    src = prep.ins[0]
    ref = act.outs[0]
    assert str(src.memref).startswith("sigf"), src.memref
    assert str(ref.memref).startswith("sig_"), ref.memref
    assert src.offset == ref.offset, (src.offset, ref.offset)
    src.memref = ref.memref
    src.memsetref = ref.memsetref


def _rewire_prep_sems(nc, preps):
    """Point each kv_writeback prep's DMA-completion update at the DMASW
    lane semaphore the tile wait pass expects.

    Tile's clock pass schedules a gen_mode==1 SWDGE prep on a DMASW lane, so
    downstream end-of-program barriers wait on that lane's semaphore; but the
    auto then_inc attach skips preps (the descriptor carries the caller's
    `sem=` instead), leaving the lane sem orphaned -> deadlock. Rewrite each
    prep's OnUpdate[0] to target its orphaned lane sem (lanes are assigned
    round-robin in emission order, so sorted lane names match prep order).
    """
    fn = nc.m.functions[0]
    updated_ids = set()
    waited = {}  # sem id -> ant_name for DMASW waits
    for block in fn.blocks:
        for ins in block.instructions:
            si = ins.sync_info
            if not si:
                continue
            for u in si.on_update:
                updated_ids.add(u.id)
            for w in si.on_wait:
                nm = getattr(w, "ant_name", None)
                if nm and str(nm).startswith("DMASW"):
                    waited[w.id] = nm
    orphans = sorted(
        (i for i in waited if i not in updated_ids),
        key=lambda i: str(waited[i]),
        reverse=True,
    )
    assert len(orphans) == len(preps), (
        f"expected {len(preps)} orphaned DMASW sems, got "
        f"{[(i, waited[i]) for i in orphans]}"
    )
    for prep, oid in zip(preps, orphans):
        upd = prep.sync_info.on_update[0]
        upd.id = oid
        upd.ant_name = waited[oid]


_NC = {}


def _get_nc():
    if "nc" not in _NC:
        _NC["nc"] = _build()
    return _NC["nc"]


def kernel(x, y, mask_x, mask_y, W):
    x = np.asarray(x, dtype=np.float32)
    y = np.asarray(y, dtype=np.float32)
    mask_x = np.asarray(mask_x, dtype=np.float32)
    mask_y = np.asarray(mask_y, dtype=np.float32)
    W = np.asarray(W, dtype=np.float32)

    xdT = np.ascontiguousarray((x * mask_x).T)  # [D, N] fp32
    xdhT = xdT.astype(NP_F8)
    xdlT = (xdT - xdhT.astype(np.float32)).astype(NP_F8)
    wT = W.T.astype(np.float32) * W_SCALE
    whT = wT.astype(NP_F8)
    wlT = (wT - whT.astype(np.float32)).astype(NP_F8)
    ydT = np.ascontiguousarray((y * mask_y).T)  # [D, M] fp32
    ydhT = ydT.astype(NP_F8)
    ydlT = (ydT - ydhT.astype(np.float32)).astype(NP_F8)

    in_maps = []
    for c in range(GRID):
        s = slice(c * N_LOC, (c + 1) * N_LOC)
        wx8T = np.empty((D, WX8), dtype=NP_F8)
        wx8T[:, 0:512] = whT
        wx8T[:, 512:1024] = xdhT[:, s][:, 0:512]
        wx8T[:, 1024:1536] = wlT
        wx8T[:, 1536:2048] = xdlT[:, s][:, 0:512]
        wx8T[:, 2048:2560] = xdhT[:, s][:, 512:1024]
        wx8T[:, 2560:3072] = xdlT[:, s][:, 512:1024]
        in_maps.append({"wx8T": wx8T, "ydhT": ydhT, "ydlT": ydlT})

    res = run_bass_kernel_spmd(_get_nc(), in_maps, list(range(8)))

    out = np.empty((N, M), dtype=np.float32)
    for c in range(GRID):
        out[c * N_LOC : (c + 1) * N_LOC, :] = res.results[c]["out"].astype(
            np.float32
        )
    return out


# revision 41
# speedup vs baseline: 1.0014x; 1.0011x over previous
"""Trainium2 Bass kernel for BilinearDiscriminator.

Computes sigmoid((x*mask_x) @ W.T @ (y*mask_y).T) for x,y [8192,512],
W [512,512] -> out [8192,8192] fp32, SPMD across 8 NeuronCores.

Sharding: 8x1 row-parallel (a 4x2 grid would duplicate mm1 on every
m-column; 8x1 halves mm1's PE work). Core c handles rows
[c*1024, (c+1)*1024) of x; W and y are replicated. Host pre-applies the
dropout masks and hi/lo-quantizes the inputs.

BOTH matmuls run in fp8-e4m3 DoubleRow perf mode (0.5 cyc/row, K=256
per matmul -> 4x the fp16 row rate) with a 3-term hi/lo decomposition
to stay inside the 2e-2 error gate: for operands A ~= Ah + Al and
B ~= Bh + Bl (each an e4m3 value plus an e4m3-quantized residual),
  A@B ~= Ah@Bh + Ah@Bl + Al@Bh          (the dropped Al@Bl is O(eps^2))
costs 6 DoubleRow matmuls (1536 cyc) per [128 x 1024] output tile vs
fp16's 4 matmuls (2048 cyc): mm2 drops 109.2us -> 81.9us and mm1
6.8us -> 5.1us of PE time. Measured rel err 3.3e-3 vs the 2e-2 gate
(fp16 chain was 8e-4). W ships pre-scaled by 2^6 -- unscaled, its lo
residual (~1.2e-3) sits below e4m3's min subnormal and flushes to
zero, which collapses the correction term -- and the xt casts unscale
by 1/64 (Copy activation scale / scalar_tensor_tensor).

y's hi/lo pair is quantized on the host (same DMA bytes as fp16 y);
xt's pair is built on-chip from mm1's fp32 PSUM: xh = Copy-activation
cast on ACT (all sigmoids share one act-table set with Copy; a dummy
1-col sigmoid at t~0 pins that set so no 1.28us mid-program reload),
xl = scalar_tensor_tensor on DVE. Cast outputs are split into small
per-kc-pair tiles (xh0a/xh0b/xl0a/xl0b/xh1/xl1): the tile framework
turns dependencies into engine-lane count waits computed from emission
order, so coarse tiles (or casts emitted too early) serialize mm2's
first tiles behind unrelated casts.

Timeline-model notes that shaped the schedule:
  - PE p-state ramp is time-based from the first PE activity; warm-up
    dummy matmuls (Pool-memset operand, ~0.94us) hold the ramp stretch
    so mm1/mm2 run at 2.4GHz from ~3.9us.
  - wx8T packs Wh|xdh0|Wl|xdl0|xdh1|xdl1 fp8 column groups so six DMA
    pieces arrive in exact consumption order of mm1's dp-outer 3-term
    loops; mm1 starts at ~3.6us off the first (Wh|xdh) piece. More
    pieces would NOT help: HWDGE costs 625ns/DMA and the exclusive DMA
    device serializes transfers, so the last y-mb0 byte (~10.2us) is a
    hard floor for mm2's streaming start (~9.6us here).
  - PSUM: psum1 = 2x[128,512] bufs + psum2 = 3x[128,1024] bufs. mm1
    packs two accumulators per psum2 tile (bank-halves as separate
    accumulation groups; sub-ranges of one 2KB zero region must close
    before a sibling group starts). mm2 rotates psum2 3-deep so the
    1024-wide sigmoid (1038ns on ACT) never back-pressures the PE.
  - mm2 streams [128n x 1024m] tiles mb-major: 6 DR matmuls ordered
    xh.yh j0/j1, xl.yh j0, xh.yl j0/j1, xl.yh j1 (latest producers
    consumed last), one sigmoid PSUM->fp16 SBUF, one store. SIG_BUFS=28
    because stores queue ~20 tiles behind the y loads on the exclusive
    DMA device.
  - Tail: mb7's last five n-chunks taper to 512-wide pieces (sigmoid
    612ns < PE 640ns per piece, so ACT never backlogs into the close);
    the last three n-chunks' stores ride SWDGE prepare+trigger
    (kv_writeback): desc-gen runs early against a decoy source, each
    trigger fires after its sigmoid, skipping HWDGE+DGE on the
    program-closing chains. _rewire_prep_sems/_patch_prep_src adapt
    the prepared stores to the Tile framework.

Measured: 94507 ns (TimelineSim) vs 122032 ns fp16 ancestor (1.29x),
rel err 3.35e-3. Roughly: 0.9 start + 9.6 mm1+casts lead-in (y-DMA
bound) + 81.9 mm2 (PE-bound, zero steady-state stalls) + 2.8 close
(trailing sigmoid + final store sem + drain barriers).
"""

import os
import sys

sys.path.insert(0, "/opt/trn_rl_repo")

import ml_dtypes
import numpy as np

import concourse.bass as bass
import concourse.mybir as mybir
import concourse.tile as tile
from concourse import bacc
from concourse.bass_utils import run_bass_kernel_spmd

P = 128
N, M, D = 8192, 8192, 512
GRID = 8
N_LOC = N // GRID  # 1024
DC = D // P  # 4 chunks of the contraction dims
MB = 1024  # mm2 column block (2 PSUM banks)
WX = D + N_LOC  # packed W|xd row length (fp16 ancestor)
WX8 = 6 * 512  # packed fp8 Wh|xdh0|Wl|xdl0|xdh1|xdl1 row length

F8 = mybir.dt.float8e4
F16 = mybir.dt.float16
F32 = mybir.dt.float32
NP_F8 = ml_dtypes.float8_e4m3

DR = mybir.MatmulPerfMode.DoubleRow
# W is shipped pre-scaled by 2^6 so its e4m3 hi AND lo parts stay in the
# normal range (unscaled, the lo residual ~1.2e-3 sits below e4m3's min
# subnormal and flushes to zero); the xt casts unscale by 1/64.
W_SCALE = 64.0
W_INV = 1.0 / W_SCALE

_SIG_BUFS = int(os.environ.get("SIG_BUFS", "28"))
_PSUM1_BUFS = int(os.environ.get("PSUM1_BUFS", "2"))
_PSUM2_BUFS = int(os.environ.get("PSUM2_BUFS", "3"))
_N_WARM = int(os.environ.get("N_WARM", "24"))


def _build():
    nc = bacc.Bacc("TRN2", target_bir_lowering=False, debug=False)

    # wx8T packs the fp8 hi/lo pairs of W^T and xd^T row-wise so few DMAs
    # deliver mm1's operands in consumption order:
    # cols [Wh 0:512 | xdh-nt0 512:1024 | Wl 1024:1536 | xdl-nt0 1536:2048
    #       | xdh-nt1 2048:2560 | xdl-nt1 2560:3072].
    wx8T = nc.dram_tensor("wx8T", [D, WX8], F8, kind="ExternalInput").ap()
    ydhT = nc.dram_tensor("ydhT", [D, M], F8, kind="ExternalInput").ap()
    ydlT = nc.dram_tensor("ydlT", [D, M], F8, kind="ExternalInput").ap()
    out = nc.dram_tensor("out", [N_LOC, M], F16, kind="ExternalOutput").ap()

    # [ (dc p) cols ] -> [ p dc cols ] so one DMA can fill a multi-d-chunk
    # SBUF tile slice in partition-major order.
    wx8T_r = wx8T.rearrange("(dc p) c -> p dc c", p=P)
    ydhT_r = ydhT.rearrange("(dc p) m -> p dc m", p=P)
    ydlT_r = ydlT.rearrange("(dc p) m -> p dc m", p=P)

    with tile.TileContext(nc) as tc:
        with (
            tc.tile_pool(name="persist", bufs=1) as persist,
            tc.tile_pool(name="sig", bufs=_SIG_BUFS) as sigp,
        ):
            wx8 = persist.tile([P, DC, WX8], F8, name="wx8")
            ydh = persist.tile([P, DC, M], F8, name="ydh")
            ydl = persist.tile([P, DC, M], F8, name="ydl")
            # xt hi/lo in fp8: [p, kc, n] with semantic k = kc*128 + p.
            # Separate tiles per mm1 column half (n 0:512 vs 512:1024) AND,
            # for nt0, per kc pair: the dep tracker is tile-granular enough
            # that one big tile would make every mm2 tile's reads wait on
            # the LAST cast, not just its own kc rows' writers.
            xh0a = persist.tile([P, 2, 512], F8, name="xh0a")
            xl0a = persist.tile([P, 2, 512], F8, name="xl0a")
            xh0b = persist.tile([P, 2, 512], F8, name="xh0b")
            xl0b = persist.tile([P, 2, 512], F8, name="xl0b")
            xh1 = persist.tile([P, DC, 512], F8, name="xh1")
            xl1 = persist.tile([P, DC, 512], F8, name="xl1")
            # Warm-up operand: a small tile memset emitted FIRST on the
            # Pool queue (its framework memsets clear by ~440ns, before the
            # DVE's ~700ns startup barrier) so the first dummy matmul issues
            # as early as possible -- that's where the PE p-state threshold
            # anchors.
            warm = persist.tile([P, P], F16, name="warm")
            nc.gpsimd.memset(warm[:], 0.0)
            # ctx index for the final kv_writeback store (the out view is
            # pre-offset to the last 512 columns, so the index is 0).
            ctx_idx = persist.tile([P, 1], mybir.dt.int32, name="ctx_idx")
            nc.gpsimd.memset(ctx_idx[:], 0)
            # Decoy source for the kv_writeback prep: same geometry as a sig
            # tile, written once at t~0 so the prep's desc-gen has no late
            # deps. After the tile passes, the prep's source AP is patched to
            # the real final sig tile (see _patch_prep_src).
            sigf = persist.tile([P, MB], F16, name="sigf")
            nc.vector.memset(sigf[:], 0.0)
            # Dummy 1-column sigmoid emitted before any other activation:
            # the act-table pass then loads the set containing BOTH Sigmoid
            # and Copy ("sigmoid_and_others") once at t~0.6us on the idle
            # ACT engine, so neither the mm1 Copy casts nor the first mm2
            # sigmoid pays the 1.28us mid-program table reload. Own scratch
            # tile: the warm-up matmuls must not chain behind the table
            # load.
            scr = persist.tile([P, 1], F16, name="scr")
            nc.vector.memset(scr[:], 0.0)
            nc.scalar.activation(
                scr[:], scr[:], mybir.ActivationFunctionType.Sigmoid,
            )

            ctx_psum1 = tc.tile_pool(name="psum1", bufs=_PSUM1_BUFS, space="PSUM")
            psum1 = ctx_psum1.__enter__()
            ctx_psum2 = tc.tile_pool(name="psum2", bufs=_PSUM2_BUFS, space="PSUM")
            psum2 = ctx_psum2.__enter__()

            # Warm-up: dependency-free dummy matmuls occupy the PE's ~3us
            # p-state ramp window so the real matmuls, gated on their first
            # DMA, run at full clock. They rotate through psum2, idle until
            # mm2.
            for i in range(_N_WARM):
                pw = psum2.tile([P, MB], F32, name="ps2")
                nc.tensor.matmul(
                    pw[:, :P],
                    lhsT=warm[:, :P],
                    rhs=warm[:, :P],
                    start=True,
                    stop=True,
                )

            # Input DMAs in priority order: per-d-pair (Wh|xdh) then
            # (Wl|xdl) pieces gate mm1-nt0's dp-outer/term-inner loop in
            # exactly consumption order; the nt1 pieces follow; then y.
            for dp in range(DC // 2):
                for c0 in (0, 1024):
                    nc.sync.dma_start(
                        out=wx8[:, 2 * dp : 2 * dp + 2, c0 : c0 + 1024],
                        in_=wx8T_r[:, 2 * dp : 2 * dp + 2, c0 : c0 + 1024],
                    )
            for dp in range(DC // 2):
                nc.sync.dma_start(
                    out=wx8[:, 2 * dp : 2 * dp + 2, 2048:3072],
                    in_=wx8T_r[:, 2 * dp : 2 * dp + 2, 2048:3072],
                )
            # y-mb0 split into 512-column pieces, hi/lo interleaved: tile0
            # consumes yh[0:512] in its first matmuls, yl[0:512] two matmuls
            # later, yh[512:1024] in its second half -- this order gets each
            # piece's completion sem in just ahead of its first consumer.
            for t8, c0 in ((ydh, 0), (ydl, 0), (ydh, 512), (ydl, 512)):
                src8 = ydhT_r if t8 is ydh else ydlT_r
                nc.sync.dma_start(
                    out=t8[:, :, c0 : c0 + 512], in_=src8[:, :, c0 : c0 + 512]
                )
            for mb in range(1, M // MB):
                nc.sync.dma_start(
                    out=ydh[:, :, mb * MB : (mb + 1) * MB],
                    in_=ydhT_r[:, :, mb * MB : (mb + 1) * MB],
                )
                nc.sync.dma_start(
                    out=ydl[:, :, mb * MB : (mb + 1) * MB],
                    in_=ydlT_r[:, :, mb * MB : (mb + 1) * MB],
                )

            # SWDGE prepares for the final stores: desc-gen runs now (Pool is
            # idle; sources are the early memsets), each transfer fires at
            # its trigger after the producing ACT. Skips a normal DMA's
            # HWDGE+DGE stages on the program-closing chains.
            import bass_rust as _br

            def wb_prep(nchunk, col0, width):
                ov = out[
                    nchunk * P : (nchunk + 1) * P, col0 : col0 + width
                ].rearrange("(b p) (o m) -> b p o m", b=1, o=1)
                # The dho dim has count 1; kv_writeback asserts
                # row_stride == dho_count * dho_stride, so patch the
                # (addressing-irrelevant) stride of that dim.
                _dims = [list(d) for d in ov.ap]
                _dims[2] = [_dims[1][0], _dims[2][1]]
                ov.ap = _br.VecI64Pair(_dims)
                in_wb = sigf[:, 0:width].rearrange(
                    "p (o b n) -> p o b n", o=1, b=1
                )
                sem = nc.alloc_semaphore(f"kvwb_dma{nchunk}_{col0}")
                return nc.gpsimd.kv_writeback(
                    ov, in_wb, ctx_idx[:],
                    prepare_only=True, sem=sem, queue_num=0,
                )

            # All preps on queue 0; the count=1 triggers fire them in FIFO
            # order (prep emission order here must match trigger emission
            # order below). The tail tapers: n5 as 2x512, n6/n7 as 4x256
            # each, all on the prepared-store path -- smaller pieces let the
            # ACT engine chase the PE through the final matmuls instead of
            # serializing ~4us of sigmoid work after them.
            n_nc = N_LOC // P
            wb_keys = [
                "act_n5_0", "act_n5_1", "act_n6_0", "act_n6_1",
                "act_n7_0", "act_n7_1",
            ]
            wb_preps = [
                wb_prep(n_nc - 3, M - 1024, 512),
                wb_prep(n_nc - 3, M - 512, 512),
                wb_prep(n_nc - 2, M - 1024, 512),
                wb_prep(n_nc - 2, M - 512, 512),
                wb_prep(n_nc - 1, M - 1024, 512),
                wb_prep(n_nc - 1, M - 512, 512),
            ]

            # mm1 (fp8 DoubleRow, 3-term like mm2): xt ~= Wh@xdh + Wl@xdh
            # + Wh@xdl. nt0 (cols 0:512) runs dp-outer (dp = contraction
            # ktile pair) x term-inner so the first matmuls need only the
            # first (Wh|xdh) DMA piece; all 4 kc accumulators live at once
            # -- kc0/kc1 in the two psum1 bufs, kc2/kc3 packed into the two
            # bank-halves of one psum2 tile.
            T1, T2, T3 = (0, 512), (1024, 512), (0, 1536)  # (lhs, rhs) col0
            psA = psum2.tile([P, MB], F32, name="ps2")
            p1a = psum1.tile([P, 512], F32, name="ps1")
            p1b = psum1.tile([P, 512], F32, name="ps1")
            nt0_ps = {
                0: (p1a, 0),
                1: (p1b, 0),
                2: (psA, 0),
                3: (psA, 512),
            }
            for dp in range(DC // 2):
                dsl = slice(2 * dp, 2 * dp + 2)
                for ti, (lb, rb) in enumerate((T1, T2, T3)):
                    for kc in range(DC):
                        ps, o = nt0_ps[kc]
                        nc.tensor.matmul(
                            ps[:, o : o + 512],
                            lhsT=wx8[:, dsl, lb + kc * P : lb + (kc + 1) * P],
                            rhs=wx8[:, dsl, rb : rb + 512],
                            start=(dp == 0 and ti == 0),
                            stop=(dp == DC // 2 - 1 and ti == 2),
                            perf_mode=DR,
                        )
            # hi/lo casts: xh = fp8(ps) as a Copy activation on the
            # otherwise-idle ACT engine, xl = fp8(ps - xh) on DVE -- the two
            # streams pipeline, halving the cast chain that gates both
            # mm1-nt1's PSUM reuse and mm2's start. kc0/kc1 first: nt1
            # reuses their psum1 bufs.
            STT = nc.vector.scalar_tensor_tensor
            MUL = mybir.AluOpType.mult
            SUB = mybir.AluOpType.subtract
            # nt0 cast schedule, balancing the serial ACT and DVE chains so
            # the last xt-low producer lands earliest: the wide fused
            # kc2/kc3 copy goes FIRST on ACT (it gates the longest sub),
            # kc0's copy runs on DVE in parallel, and the subs chain on DVE
            # in c-completion order.
            xh0_23 = xh0b[:, :, :].rearrange("p a b -> p (a b)")
            xl0_23 = xl0b[:, :, :].rearrange("p a b -> p (a b)")
            nc.scalar.activation(
                xh0_23, psA[:, 0:1024], mybir.ActivationFunctionType.Copy,
                scale=W_INV,
            )
            nc.vector.tensor_scalar_mul(xh0a[:, 0, :], p1a[:], W_INV)
            STT(xl0a[:, 0, :], p1a[:], W_INV, xh0a[:, 0, :], MUL, SUB)
            nc.scalar.activation(
                xh0a[:, 1, :], p1b[:],
                mybir.ActivationFunctionType.Copy, scale=W_INV,
            )
            STT(xl0_23, psA[:, 0:1024], W_INV, xh0_23, MUL, SUB)
            STT(xl0a[:, 1, :], p1b[:], W_INV, xh0a[:, 1, :], MUL, SUB)

            # mm1, nt1 (cols 512:1024), kc-outer, as 384- then 128-column
            # sub-chunks matching the split DMAs above. The two sub-chunks
            # pack into ONE [128,512] accumulator range per kc (cols [0:384]
            # and [384:512] are separate accumulation groups) which maps
            # contiguously onto xt cols 512:1024, so each kc needs one
            # copy+sub. kc0/kc1 take the second psum2 tile's halves (free
            # since the warm-up dummies); kc2/kc3 reuse psum1 whose nt0
            # (kc0/kc1) casts complete earliest.
            psB = psum2.tile([P, MB], F32, name="ps2")
            p1c = psum1.tile([P, 512], F32, name="ps1")
            p1d = psum1.tile([P, 512], F32, name="ps1")
            nt1_ps = {
                0: (psB, 0),
                1: (psB, 512),
                2: (p1c, 0),
                3: (p1d, 0),
            }
            NT1_T = ((0, 2048), (1024, 2048), (0, 2560))  # (lhs, rhs) col0
            R0, R1 = (0, 384, 0), (384, 128, 384)

            def nt1_mm(kc, rng, dps):
                ps, po = nt1_ps[kc]
                xo, w, o = rng
                for dp in dps:
                    dsl = slice(2 * dp, 2 * dp + 2)
                    for ti, (lb, rb) in enumerate(NT1_T):
                        nc.tensor.matmul(
                            ps[:, po + o : po + o + w],
                            lhsT=wx8[:, dsl, lb + kc * P : lb + (kc + 1) * P],
                            rhs=wx8[:, dsl, rb + xo : rb + xo + w],
                            start=(dp == 0 and ti == 0),
                            stop=(dp == DC // 2 - 1 and ti == 2),
                            perf_mode=DR,
                        )

            # Group order absorbs the second nt1 DMA piece's arrival (kc0/
            # kc1's dp0 work runs first) and the psum1 WAR gates (kc2/kc3
            # wait the nt0-kc0/kc1 subs). Each (kc, range) accumulation
            # group still closes before its sibling range starts (shared
            # PSUM zero region).
            nt1_mm(0, R0, [0]); nt1_mm(1, R0, [0])
            nt1_mm(0, R0, [1]); nt1_mm(0, R1, [0, 1])
            nt1_mm(1, R0, [1]); nt1_mm(1, R1, [0, 1])
            for kc in (2, 3):
                nt1_mm(kc, R0, [0]); nt1_mm(kc, R0, [1])
                nt1_mm(kc, R1, [0, 1])
            # nt1 casts are EMITTED after mm2's first n-chunk (below):
            # the framework turns deps into engine-lane count waits computed
            # from emission order, so casts emitted here would inflate the
            # first mm2 tile's DVE wait to include them.
            def nt1_casts():
                xh1_01 = xh1[:, 0:2, :].rearrange("p a b -> p (a b)")
                nc.scalar.activation(
                    xh1_01, psB[:, 0:1024],
                    mybir.ActivationFunctionType.Copy, scale=W_INV,
                )
                xl1_01 = xl1[:, 0:2, :].rearrange("p a b -> p (a b)")
                STT(xl1_01, psB[:, 0:1024], W_INV, xh1_01, MUL, SUB)
                # kc2/kc3 copies on DVE, not ACT: on ACT the scheduler
                # runs them (ready early) ahead of mm2's first sigmoids,
                # and the n1 tile's hoisted ACT-count wait then spans them.
                for kc in (2, 3):
                    ps, po = nt1_ps[kc]
                    nc.vector.tensor_scalar_mul(
                        xh1[:, kc, :], ps[:, po : po + 512], W_INV
                    )
                    STT(xl1[:, kc, :], ps[:, po : po + 512], W_INV,
                        xh1[:, kc, :], MUL, SUB)

            # mm2 + sigmoid + store, streaming mb-major over y blocks.
            # Each tile: 6 fp8 DoubleRow matmuls (K=256 each) accumulating
            # xh@yh + xh@yl + xl@yh into PSUM fp32. Term order puts yl- and
            # xl-dependent matmuls later to relax their producers' deadlines.
            n_mb = M // MB
            handles = {}

            # Deferred act+store emission: the framework converts deps into
            # engine-lane count waits computed at emission time, so a tile's
            # matmuls conservatively wait on ALL previously-emitted ACT ops.
            # Emitting each tile's sigmoid+store two tiles late keeps every
            # preceding sigmoid out of the next tiles' count windows (they
            # are all long done by then at runtime).
            deferred = []

            def mm2_tile(mb, nchunk, width, coff, pool=None, wb_key=None,
                         ps=None, psoff=0, defer=False):
                if ps is None:
                    if pool is None:
                        ps = psum2.tile([P, MB], F32, name="ps2")
                    else:
                        ps = pool.tile([P, 512], F32, name="ps1")
                grp = min(width, 512)
                if nchunk < 4:
                    # nt0 halves: per-j tiles with local ktile index 0:2.
                    xh_j = {0: (xh0a, slice(0, 2)), 1: (xh0b, slice(0, 2))}
                    xl_j = {0: (xl0a, slice(0, 2)), 1: (xl0b, slice(0, 2))}
                else:
                    xh_j = {j: (xh1, slice(2 * j, 2 * j + 2)) for j in (0, 1)}
                    xl_j = {j: (xl1, slice(2 * j, 2 * j + 2)) for j in (0, 1)}
                nsl = slice((nchunk % 4) * P, (nchunk % 4 + 1) * P)
                sig = sigp.tile([P, MB], F16, name="sig")
                # mt-outer: the first tile's first-half matmuls run before
                # its second-half ones, covering the second y half-block's
                # slightly later arrival.
                for mt in range(width // grp):
                    msl = slice(
                        mb * MB + coff + mt * grp, mb * MB + coff + (mt + 1) * grp
                    )
                    osl = slice(mt * grp, (mt + 1) * grp)
                    # Order: xh.yh (j0,j1), xl.yh j0, xh.yl (j0,j1),
                    # xl.yh j1 LAST -- the kc2/3 low-part cast (s23) is the
                    # latest xt producer, so its consumer goes last.
                    seq = [
                        (xh_j, ydh, 0), (xh_j, ydh, 1), (xl_j, ydh, 0),
                        (xh_j, ydl, 0), (xh_j, ydl, 1), (xl_j, ydh, 1),
                    ]
                    for i, (lhsd, rhs, j) in enumerate(seq):
                        lhs, ksl = lhsd[j]
                        nc.tensor.matmul(
                            ps[:, psoff + osl.start : psoff + osl.stop],
                            lhsT=lhs[:, ksl, nsl],
                            rhs=rhs[:, 2 * j : 2 * j + 2, msl],
                            start=(i == 0),
                            stop=(i == len(seq) - 1),
                            perf_mode=DR,
                        )
                def finish():
                    act = nc.scalar.activation(
                        sig[:, :width],
                        ps[:, psoff : psoff + width],
                        mybir.ActivationFunctionType.Sigmoid,
                    )
                    if wb_key is not None:
                        # Prepared-store path: fire this tile's SWDGE
                        # descriptors (signals_writable carries the WAW dep
                        # on the ACT above; the matching prep is repointed
                        # at this sig tile by _patch_prep_src).
                        handles[wb_key] = act
                        nc.gpsimd.trigger_dma(
                            count=1, queue_num=0,
                            signals_writable=[sig[:, :width]],
                        )
                    else:
                        nc.sync.dma_start(
                            out=out[
                                nchunk * P : (nchunk + 1) * P,
                                mb * MB + coff : mb * MB + coff + width,
                            ],
                            in_=sig[:, :width],
                        )

                if defer:
                    deferred.append(finish)
                    while len(deferred) > 2:
                        deferred.pop(0)()
                else:
                    while deferred:
                        deferred.pop(0)()
                    finish()

            # Tail taper: the last four n-chunks of the last y block run as
            # progressively smaller pieces, alternating psum2/psum1 for a
            # 5-deep effective rotation; n5-n7 ride the prepared-store path.
            for mb in range(n_mb):
                for nchunk in range(n_nc):
                    if mb == 0 and nchunk == 0:
                        # First tile as two 512 halves SHARING one psum2
                        # tile: the framework hoists a tile's waits onto its
                        # first instruction, so a 1024-wide tile0 would idle
                        # on the last y-mb0 piece that only its second half
                        # needs; sharing one tile keeps the psum2 rotation
                        # aligned so (mb0,n1) lands on psA (casts done
                        # early), not psB (casts late).
                        ps0 = psum2.tile([P, MB], F32, name="ps2")
                        mm2_tile(mb, nchunk, 512, 0, ps=ps0, defer=True)
                        mm2_tile(mb, nchunk, 512, 512, ps=ps0, psoff=512,
                                 defer=True)
                    elif mb == 0 and nchunk == 1:
                        mm2_tile(mb, nchunk, MB, 0, defer=True)
                        # nt1 casts emitted only now: early enough to precede
                        # (mb0,n2) which reuses psB's psum buf, late enough
                        # that neither tile0's nor this tile's hoisted
                        # DVE-count waits include the nt1 subs.
                        nt1_casts()
                    elif mb == n_mb - 1 and nchunk in (n_nc - 5, n_nc - 4):
                        # n3/n4 as 2x512 normal tiles: steps the ACT chain
                        # down from 1038ns sigmoids before the prepared tail
                        # so no backlog carries into the close.
                        mm2_tile(mb, nchunk, 512, 0)
                        mm2_tile(mb, nchunk, 512, 512, pool=psum1)
                    elif mb == n_mb - 1 and nchunk >= n_nc - 3 and nchunk < n_nc - 1:
                        # Uniform 512-wide prepared pieces from n5 on: each
                        # sigmoid (612ns) is shorter than its piece's PE time
                        # (640ns), so the ACT chain never backlogs into the
                        # close (a 1024-wide n5 sigmoid did).
                        nk = f"n{5 + (nchunk - (n_nc - 3))}"
                        mm2_tile(mb, nchunk, 512, 0, wb_key=f"act_{nk}_0")
                        mm2_tile(mb, nchunk, 512, 512, pool=psum1,
                                 wb_key=f"act_{nk}_1")
                    elif mb == n_mb - 1 and nchunk == n_nc - 1:
                        mm2_tile(mb, nchunk, 512, 0, wb_key="act_n7_0")
                        mm2_tile(mb, nchunk, 512, 512, pool=psum1,
                                 wb_key="act_n7_1")
                    else:
                        mm2_tile(mb, nchunk, MB, 0, defer=True)

            ctx_psum2.__exit__(None, None, None)
            ctx_psum1.__exit__(None, None, None)

    _rewire_prep_sems(nc, [p.ins for p in wb_preps])
    for key, p in zip(wb_keys, wb_preps):
        _patch_prep_src(p.ins, handles[key].ins)
    nc.compile()
    return nc


def _patch_prep_src(prep, act):
    """Repoint the kv_writeback prep's source from the sigf decoy to the
    real final sig tile (same geometry; only the memory ref differs)."""
    src = prep.ins[0]
    ref = act.outs[0]
    assert str(src.memref).startswith("sigf"), src.memref
    assert str(ref.memref).startswith("sig_"), ref.memref
    assert src.offset == ref.offset, (src.offset, ref.offset)
    src.memref = ref.memref
    src.memsetref = ref.memsetref


def _rewire_prep_sems(nc, preps):
    """Point each kv_writeback prep's DMA-completion update at the DMASW
    lane semaphore the tile wait pass expects.

    Tile's clock pass schedules a gen_mode==1 SWDGE prep on a DMASW lane, so
    downstream end-of-program barriers wait on that lane's semaphore; but the
    auto then_inc attach skips preps (the descriptor carries the caller's
    `sem=` instead), leaving the lane sem orphaned -> deadlock. Rewrite each
    prep's OnUpdate[0] to target its orphaned lane sem (lanes are assigned
    round-robin in emission order, so sorted lane names match prep order).
    """
    fn = nc.m.functions[0]
    updated_ids = set()
    waited = {}  # sem id -> ant_name for DMASW waits
    for block in fn.blocks:
        for ins in block.instructions:
            si = ins.sync_info
            if not si:
                continue
            for u in si.on_update:
                updated_ids.add(u.id)
            for w in si.on_wait:
                nm = getattr(w, "ant_name", None)
                if nm and str(nm).startswith("DMASW"):
                    waited[w.id] = nm
    orphans = sorted(
        (i for i in waited if i not in updated_ids),
        key=lambda i: str(waited[i]),
        reverse=True,
    )
    assert len(orphans) == len(preps), (
        f"expected {len(preps)} orphaned DMASW sems, got "
        f"{[(i, waited[i]) for i in orphans]}"
    )
    for prep, oid in zip(preps, orphans):
        upd = prep.sync_info.on_update[0]
        upd.id = oid
        upd.ant_name = waited[oid]


_NC = {}


def _get_nc():
    if "nc" not in _NC:
        _NC["nc"] = _build()
    return _NC["nc"]


def kernel(x, y, mask_x, mask_y, W):
    x = np.asarray(x, dtype=np.float32)
    y = np.asarray(y, dtype=np.float32)
    mask_x = np.asarray(mask_x, dtype=np.float32)
    mask_y = np.asarray(mask_y, dtype=np.float32)
    W = np.asarray(W, dtype=np.float32)

    xdT = np.ascontiguousarray((x * mask_x).T)  # [D, N] fp32
    xdhT = xdT.astype(NP_F8)
    xdlT = (xdT - xdhT.astype(np.float32)).astype(NP_F8)
    wT = W.T.astype(np.float32) * W_SCALE
    whT = wT.astype(NP_F8)
    wlT = (wT - whT.astype(np.float32)).astype(NP_F8)
    ydT = np.ascontiguousarray((y * mask_y).T)  # [D, M] fp32
    ydhT = ydT.astype(NP_F8)
    ydlT = (ydT - ydhT.astype(np.float32)).astype(NP_F8)

    in_maps = []
    for c in range(GRID):
        s = slice(c * N_LOC, (c + 1) * N_LOC)
        wx8T = np.empty((D, WX8), dtype=NP_F8)
        wx8T[:, 0:512] = whT
        wx8T[:, 512:1024] = xdhT[:, s][:, 0:512]
        wx8T[:, 1024:1536] = wlT
        wx8T[:, 1536:2048] = xdlT[:, s][:, 0:512]
        wx8T[:, 2048:2560] = xdhT[:, s][:, 512:1024]
        wx8T[:, 2560:3072] = xdlT[:, s][:, 512:1024]
        in_maps.append({"wx8T": wx8T, "ydhT": ydhT, "ydlT": ydlT})

    res = run_bass_kernel_spmd(_get_nc(), in_maps, list(range(8)))

    out = np.empty((N, M), dtype=np.float32)
    for c in range(GRID):
        out[c * N_LOC : (c + 1) * N_LOC, :] = res.results[c]["out"].astype(
            np.float32
        )
    return out
